# revision 3
# baseline (speedup 1.0000x reference)
"""Relational GAT message-passing kernel for 8 Trainium2 NeuronCores.

Strategy (zero-collective, 1D row partitioning):
  - Edges are sharded by subject-node range: core c owns all edges whose
    edge_sub falls in [c*N/8, (c+1)*N/8). Segment rows (sub + pred*N) for
    those subjects live entirely on that core, so segment-max/sum and the
    scatter-add need no cross-core reduction at all.
  - Within a core, edges are grouped into "windows" = (pred, 128-subject
    block). Each window's edges are padded to TPW tiles of 128 edge slots.
  - Per tile: indirect-DMA gather x[obj] rows, project queries/values with
    block-diagonal per-relation weights on TensorE; subject-side keys are
    selected from the window's key projection via a one-hot selector matmul
    (built with an iota compare on VectorE) - no gather needed for the
    subject side. Segment-sum of both the messages and the softmax
    denominators happens in one selector matmul accumulated in PSUM.
  - Softmax skips the segment-max subtraction: dot products here are
    z-scale ~2 (x ~ N(0,1), weights uniform(+-1/sqrt(S))), exp() is safe in
    f32 and the result is mathematically identical.
  - Finale: per 128-node block, unify matmuls accumulate over the 4
    relations in PSUM, ReLU, DMA out. Host concatenates the 8 slices.
"""
import sys

sys.path.insert(0, "/opt/trn_rl_repo")

import numpy as np

N = 50000
R = 4
EMB = 128
H = 4
S = 32
C = 8
NPC = N // C            # 6250 subjects per core
WROWS = 128             # segment rows per window
NWPP = (NPC + WROWS - 1) // WROWS   # windows per relation  (49)
NWIN = R * NWPP         # windows per core (196)
P = 128


def _split_waits(nc, mybir, max_waits=1):
    """This walrus build encodes at most one sync-wait per instruction.
    Hoist excess waits onto NoOp instructions inserted just before."""
    n_split = 0
    for fn in nc.m.functions:
        for block in fn.blocks:
            new_list = []
            for inst in block.instructions:
                si = inst.sync_info
                if si is not None and len(si.on_wait) > max_waits:
                    waits = list(si.on_wait)
                    for w in waits[:-max_waits]:
                        nop = mybir.InstNoOp(
                            name=nc.get_next_instruction_name(),
                            text_hint="waitsplit",
                        )
                        nop.engine = inst.engine
                        nop.sync_info = mybir.SyncInfo(on_wait=[w], on_update=[])
                        new_list.append(nop)
                        n_split += 1
                    inst.sync_info = mybir.SyncInfo(
                        on_wait=waits[-max_waits:], on_update=list(si.on_update)
                    )
                new_list.append(inst)
            block.instructions[:] = new_list
    return n_split


def build_program(n, r, npc, nwpp, tpw):
    """Build the SPMD Bass program (identical for all cores)."""
    import concourse.bass as bass
    import concourse.tile as tile
    from concourse import mybir

    f32 = mybir.dt.float32
    i32 = mybir.dt.int32
    Alu = mybir.AluOpType
    Act = mybir.ActivationFunctionType
    Ax = mybir.AxisListType

    nwin = r * nwpp
    nt = nwin * tpw
    xt_cols = nwpp * P

    nc = bass.Bass()
    x_d = nc.dram_tensor("x", [n, EMB], f32, kind="ExternalInput")
    xt_d = nc.dram_tensor("xt", [EMB, xt_cols], f32, kind="ExternalInput")
    wk_d = nc.dram_tensor("wk", [EMB, r, EMB], f32, kind="ExternalInput")
    wq_d = nc.dram_tensor("wq", [EMB, r, EMB], f32, kind="ExternalInput")
    wv_d = nc.dram_tensor("wv", [EMB, r, EMB], f32, kind="ExternalInput")
    ut_d = nc.dram_tensor("ut", [EMB, r, EMB], f32, kind="ExternalInput")
    obj_d = nc.dram_tensor("obj", [P, nt], i32, kind="ExternalInput")
    rid_d = nc.dram_tensor("rid", [P, nt], f32, kind="ExternalInput")
    iota_d = nc.dram_tensor("iota", [P, P], f32, kind="ExternalInput")
    id_d = nc.dram_tensor("ident", [P, P], f32, kind="ExternalInput")
    out_d = nc.dram_tensor("out", [npc, EMB], f32, kind="ExternalOutput")

    def bc3(ap2, sz):
        # [P, k] -> [P, k, sz] free-dim broadcast (stride 0)
        return bass.AP(tensor=ap2.tensor, offset=ap2.offset,
                       ap=[ap2.ap[0], ap2.ap[1], [0, sz]])

    with tile.TileContext(nc) as tc, \
         tc.tile_pool(name="const", bufs=1) as constp, \
         tc.tile_pool(name="sbw", bufs=3) as sbw, \
         tc.tile_pool(name="sbt", bufs=3) as sbt, \
         tc.tile_pool(name="psA", bufs=4, space="PSUM") as psA, \
         tc.tile_pool(name="psU", bufs=2, space="PSUM") as psU, \
         tc.tile_pool(name="psO", bufs=2, space="PSUM") as psO:

        xt_t = constp.tile([P, xt_cols], f32)
        nc.sync.dma_start(out=xt_t[:], in_=xt_d[:])
        wk_t = constp.tile([P, r, EMB], f32)
        nc.sync.dma_start(out=wk_t[:], in_=wk_d[:])
        wq_t = constp.tile([P, r, EMB], f32)
        nc.sync.dma_start(out=wq_t[:], in_=wq_d[:])
        wv_t = constp.tile([P, r, EMB], f32)
        nc.sync.dma_start(out=wv_t[:], in_=wv_d[:])
        ut_t = constp.tile([P, r, EMB], f32)
        nc.sync.dma_start(out=ut_t[:], in_=ut_d[:])
        obj_t = constp.tile([P, nt], i32)
        nc.sync.dma_start(out=obj_t[:], in_=obj_d[:])
        rid_t = constp.tile([P, nt], f32)
        nc.sync.dma_start(out=rid_t[:], in_=rid_d[:])
        iota_t = constp.tile([P, P], f32)
        nc.sync.dma_start(out=iota_t[:], in_=iota_d[:])
        id_t = constp.tile([P, P], f32)
        nc.sync.dma_start(out=id_t[:], in_=id_d[:])
        eps_t = constp.tile([P, 1], f32)
        nc.vector.memset(eps_t[:], 1e-30)
        aggnt = constp.tile([P, nwin, P], f32)

        for w in range(nwin):
            pred = w // nwpp
            sb = w % nwpp
            base = sb * P

            # key projection for this window's 128 subjects
            kwin_ps = psO.tile([P, P], f32, space="PSUM", tag="po")
            nc.tensor.matmul(out=kwin_ps[:], lhsT=xt_t[:, base:base + P],
                             rhs=wk_t[:, pred, :], start=True, stop=True)
            kwin = sbw.tile([P, P], f32, tag="kwin")
            nc.scalar.activation(out=kwin[:], in_=kwin_ps[:], func=Act.Copy,
                                 scale=1.0)

            aggu_ps = psU.tile([P, P + H], f32, space="PSUM", tag="pu")
            for k in range(tpw):
                t = w * tpw + k
                # gather x[obj] for this tile's 128 edges
                xg = sbt.tile([P, P], f32, tag="xg")
                nc.gpsimd.indirect_dma_start(
                    out=xg[:], out_offset=None, in_=x_d[:],
                    in_offset=bass.IndirectOffsetOnAxis(
                        ap=obj_t[:, t:t + 1], axis=0))
                # transpose -> [emb, edges]
                xgT_ps = psA.tile([P, P], f32, space="PSUM", tag="pa")
                nc.tensor.transpose(out=xgT_ps[:], in_=xg[:], identity=id_t[:])
                xgT = sbt.tile([P, P], f32, tag="xgT")
                nc.scalar.activation(out=xgT[:], in_=xgT_ps[:], func=Act.Copy,
                                     scale=1.0)
                # one-hot selector G^T[e, i] = (rid_rel[e] == i)
                GT = sbt.tile([P, P], f32, tag="GT")
                nc.vector.tensor_tensor(
                    out=GT[:], in0=rid_t[:, t:t + 1].to_broadcast([P, P]),
                    in1=iota_t[:], op=Alu.is_equal)
                G_ps = psA.tile([P, P], f32, space="PSUM", tag="pa")
                nc.tensor.transpose(out=G_ps[:], in_=GT[:], identity=id_t[:])
                G = sbt.tile([P, P], f32, tag="G")
                nc.vector.tensor_copy(out=G[:], in_=G_ps[:])
                # projections
                sq_ps = psA.tile([P, P], f32, space="PSUM", tag="pa")
                nc.tensor.matmul(out=sq_ps[:], lhsT=xgT[:],
                                 rhs=wq_t[:, pred, :], start=True, stop=True)
                sq = sbt.tile([P, P], f32, tag="sq")
                nc.vector.tensor_copy(out=sq[:], in_=sq_ps[:])
                sv_ps = psA.tile([P, P], f32, space="PSUM", tag="pa")
                nc.tensor.matmul(out=sv_ps[:], lhsT=xgT[:],
                                 rhs=wv_t[:, pred, :], start=True, stop=True)
                sk_ps = psA.tile([P, P], f32, space="PSUM", tag="pa")
                nc.tensor.matmul(out=sk_ps[:], lhsT=G[:], rhs=kwin[:],
                                 start=True, stop=True)
                # dot product per head, then exp
                prod = sbt.tile([P, P], f32, tag="prod")
                nc.vector.tensor_tensor(out=prod[:], in0=sk_ps[:], in1=sq[:],
                                        op=Alu.mult)
                dot = sbt.tile([P, H], f32, tag="dot")
                nc.vector.tensor_reduce(
                    out=dot[:], in_=prod[:].rearrange("p (h s) -> p h s", h=H),
                    axis=Ax.X, op=Alu.add)
                msg = sbt.tile([P, P + H], f32, tag="msg")
                nc.scalar.activation(out=msg[:, P:P + H], in_=dot[:],
                                     func=Act.Exp, scale=1.0)
                # msg = ex * v  (broadcast ex per head)
                nc.vector.tensor_tensor(
                    out=msg[:, 0:P].rearrange("p (h s) -> p h s", h=H),
                    in0=sv_ps[:].rearrange("p (h s) -> p h s", h=H),
                    in1=bc3(msg[:, P:P + H], S),
                    op=Alu.mult)
                # segment-sum of [msg | ex] into this window's accumulator
                nc.tensor.matmul(out=aggu_ps[:], lhsT=GT[:], rhs=msg[:],
                                 start=(k == 0), stop=(k == tpw - 1))

            # normalize rows: agg / (segsum + eps)
            recip = sbw.tile([P, H], f32, tag="recip")
            nc.scalar.activation(out=recip[:], in_=aggu_ps[:, P:P + H],
                                 func=Act.Copy, bias=1e-30, scale=1.0)
            nc.vector.reciprocal(out=recip[:], in_=recip[:])
            aggn = sbw.tile([P, P], f32, tag="aggn")
            nc.vector.tensor_tensor(
                out=aggn[:].rearrange("p (h s) -> p h s", h=H),
                in0=aggu_ps[:, 0:P].rearrange("p (h s) -> p h s", h=H),
                in1=bc3(recip[:], S),
                op=Alu.mult)
            aggnT_ps = psO.tile([P, P], f32, space="PSUM", tag="po")
            nc.tensor.transpose(out=aggnT_ps[:], in_=aggn[:], identity=id_t[:])
            nc.scalar.activation(out=aggnt[:, w, :], in_=aggnT_ps[:],
                                 func=Act.Copy, scale=1.0)

        # finale: out[n, i] = relu(sum_r aggn[r block] @ unify[r]^T)
        for sb in range(nwpp):
            nrows = min(P, npc - sb * P)
            o_ps = psU.tile([P, P], f32, space="PSUM", tag="pu")
            for pred in range(r):
                nc.tensor.matmul(out=o_ps[:], lhsT=aggnt[:, pred * nwpp + sb, :],
                                 rhs=ut_t[:, pred, :],
                                 start=(pred == 0), stop=(pred == r - 1))
            o_sb = sbw.tile([P, P], f32, tag="osb")
            nc.scalar.activation(out=o_sb[:], in_=o_ps[:], func=Act.Relu,
                                 scale=1.0)
            nc.sync.dma_start(out=out_d[sb * P: sb * P + nrows, :],
                              in_=o_sb[:nrows, :])

    _split_waits(nc, mybir)
    return nc


def host_prep(x, tokeys, toqueries, tovals, unify, edge_sub, edge_pred,
              edge_obj, n, r, c, npc, nwpp):
    """Shard + pack edges per core; pre-arrange weights. Returns
    (in_maps, tpw)."""
    x = np.ascontiguousarray(np.asarray(x, dtype=np.float32))
    tokeys = np.asarray(tokeys, dtype=np.float32)
    toqueries = np.asarray(toqueries, dtype=np.float32)
    tovals = np.asarray(tovals, dtype=np.float32)
    unify = np.asarray(unify, dtype=np.float32)
    sub = np.asarray(edge_sub).astype(np.int64)
    pred = np.asarray(edge_pred).astype(np.int64)
    obj = np.asarray(edge_obj).astype(np.int64)

    nwin = r * nwpp
    h, s = tokeys.shape[1], tokeys.shape[2]

    def blockdiag(wr):  # [r,h,s,s] -> [emb(j_in), r, emb(i_out)]
        bd = np.zeros((r, EMB, EMB), dtype=np.float32)
        for rr in range(r):
            for hh in range(h):
                # out[n,h,i] = sum_j W[r,h,i,j] x[n,h,j] -> bd[(h,j),(h,i)]=W[r,h,i,j]
                bd[rr, hh * s:(hh + 1) * s, hh * s:(hh + 1) * s] = wr[rr, hh].T
        return np.ascontiguousarray(bd.transpose(1, 0, 2))

    wk_host = blockdiag(tokeys)
    wq_host = blockdiag(toqueries)
    wv_host = blockdiag(tovals)
    ut_host = np.ascontiguousarray(unify.transpose(2, 0, 1))  # [j, r, i]
    iota_host = np.ascontiguousarray(
        np.broadcast_to(np.arange(P, dtype=np.float32), (P, P)))
    id_host = np.eye(P, dtype=np.float32)

    core = sub // npc
    subloc = sub - core * npc
    win = pred * nwpp + subloc // WROWS
    ridrel = (subloc % WROWS).astype(np.float32)

    # per-core packing
    percore = []
    tpw = 1
    for cc in range(c):
        m = core == cc
        wc = win[m]
        order = np.argsort(wc, kind="stable")
        wc = wc[order]
        rr = ridrel[m][order]
        ob = obj[m][order]
        counts = np.bincount(wc, minlength=nwin)
        tpw = max(tpw, int(np.ceil(counts.max() / P)))
        starts = np.zeros(nwin, dtype=np.int64)
        starts[1:] = np.cumsum(counts)[:-1]
        rank = np.arange(len(wc)) - starts[wc]
        percore.append((cc, wc, rr, ob, rank))

    nt = nwin * tpw
    in_maps = []
    for cc, wc, rr, ob, rank in percore:
        slot = wc * (tpw * P) + rank
        obj_arr = np.zeros(nt * P, dtype=np.int32)
        rid_arr = np.full(nt * P, -1.0, dtype=np.float32)
        obj_arr[slot] = ob.astype(np.int32)
        rid_arr[slot] = rr
        obj_host = np.ascontiguousarray(obj_arr.reshape(nt, P).T)
        rid_host = np.ascontiguousarray(rid_arr.reshape(nt, P).T)
        xt_host = np.zeros((EMB, nwpp * P), dtype=np.float32)
        xt_host[:, :npc] = x[cc * npc:(cc + 1) * npc].T
        in_maps.append({
            "x": x, "xt": xt_host,
            "wk": wk_host, "wq": wq_host, "wv": wv_host, "ut": ut_host,
            "obj": obj_host, "rid": rid_host,
            "iota": iota_host, "ident": id_host,
        })
    return in_maps, tpw


_CACHE = {}


def _get_program(n, r, npc, nwpp, tpw):
    key = (n, r, npc, nwpp, tpw)
    if key not in _CACHE:
        _CACHE[key] = build_program(n, r, npc, nwpp, tpw)
    return _CACHE[key]


def kernel(x, tokeys, toqueries, tovals, unify, edge_sub, edge_pred, edge_obj):
    from concourse.bass_utils import run_bass_kernel_spmd

    in_maps, tpw = host_prep(x, tokeys, toqueries, tovals, unify,
                             edge_sub, edge_pred, edge_obj,
                             N, R, C, NPC, NWPP)
    nc = _get_program(N, R, NPC, NWPP, tpw)
    res = run_bass_kernel_spmd(nc, in_maps, list(range(C)))
    out = np.concatenate([res.results[c]["out"] for c in range(C)], axis=0)
    return np.ascontiguousarray(out, dtype=np.float32)


# revision 7
# speedup vs baseline: 84.4659x; 84.4659x over previous
"""Relational GAT message-passing kernel for 8 Trainium2 NeuronCores.

Strategy (zero-collective, 1D row partitioning):
  - Edges are sharded by subject-node range: core c owns all edges whose
    edge_sub falls in [c*N/8, (c+1)*N/8). Segment rows (sub + pred*N) for
    those subjects live entirely on that core, so segment-max/sum and the
    scatter-add need no cross-core reduction at all.
  - Within a core, edges are grouped into "windows" = (pred, 128-subject
    block). Each window's edges are padded to TPW tiles of 128 edge slots.
  - Per tile: indirect-DMA gather x[obj] rows, project queries/values with
    block-diagonal per-relation weights on TensorE; subject-side keys are
    selected from the window's key projection via a one-hot selector matmul
    (built with an iota compare on VectorE) - no gather needed for the
    subject side. Segment-sum of both the messages and the softmax
    denominators happens in one selector matmul accumulated in PSUM.
  - Softmax skips the segment-max subtraction: dot products here are
    z-scale ~2 (x ~ N(0,1), weights uniform(+-1/sqrt(S))), exp() is safe in
    f32 and the result is mathematically identical.
  - Finale: per 128-node block, unify matmuls accumulate over the 4
    relations in PSUM, ReLU, DMA out. Host concatenates the 8 slices.
"""
import sys

sys.path.insert(0, "/opt/trn_rl_repo")

import numpy as np

N = 50000
R = 4
EMB = 128
H = 4
S = 32
C = 8
NPC = N // C            # 6250 subjects per core
WROWS = 128             # segment rows per window
NWPP = (NPC + WROWS - 1) // WROWS   # windows per relation  (49)
NWIN = R * NWPP         # windows per core (196)
P = 128


def _split_waits(nc, mybir, max_waits=1):
    """This walrus build encodes at most one sync-wait per instruction.
    Hoist excess waits onto NoOp instructions inserted just before."""
    n_split = 0
    for fn in nc.m.functions:
        for block in fn.blocks:
            new_list = []
            for inst in block.instructions:
                si = inst.sync_info
                if si is not None and len(si.on_wait) > max_waits:
                    waits = list(si.on_wait)
                    for w in waits[:-max_waits]:
                        nop = mybir.InstNoOp(
                            name=nc.get_next_instruction_name(),
                            text_hint="waitsplit",
                        )
                        nop.engine = inst.engine
                        nop.sync_info = mybir.SyncInfo(on_wait=[w], on_update=[])
                        new_list.append(nop)
                        n_split += 1
                    inst.sync_info = mybir.SyncInfo(
                        on_wait=waits[-max_waits:], on_update=list(si.on_update)
                    )
                new_list.append(inst)
            block.instructions[:] = new_list
    return n_split


def build_program(n, r, npc, nwpp, tpw, loop_iters=1):
    """Build the SPMD Bass program (identical for all cores).

    loop_iters > 1 wraps the compute body in an on-device For loop so the
    whole kernel repeats inside one dispatch (benchmarking only)."""
    import contextlib
    import concourse.bass as bass
    import concourse.tile as tile
    from concourse import mybir

    f32 = mybir.dt.float32
    i32 = mybir.dt.int32
    Alu = mybir.AluOpType
    Act = mybir.ActivationFunctionType
    Ax = mybir.AxisListType

    nwin = r * nwpp
    nt = nwin * tpw
    xt_cols = nwpp * P

    nc = bass.Bass()
    x_d = nc.dram_tensor("x", [n, EMB], f32, kind="ExternalInput")
    xt_d = nc.dram_tensor("xt", [EMB, xt_cols], f32, kind="ExternalInput")
    wk_d = nc.dram_tensor("wk", [EMB, r, EMB], f32, kind="ExternalInput")
    wq_d = nc.dram_tensor("wq", [EMB, r, EMB], f32, kind="ExternalInput")
    wv_d = nc.dram_tensor("wv", [EMB, r, EMB], f32, kind="ExternalInput")
    ut_d = nc.dram_tensor("ut", [EMB, r, EMB], f32, kind="ExternalInput")
    obj_d = nc.dram_tensor("obj", [P, nt], i32, kind="ExternalInput")
    rid_d = nc.dram_tensor("rid", [P, nt], f32, kind="ExternalInput")
    iota_d = nc.dram_tensor("iota", [P, P], f32, kind="ExternalInput")
    id_d = nc.dram_tensor("ident", [P, P], f32, kind="ExternalInput")
    out_d = nc.dram_tensor("out", [npc, EMB], f32, kind="ExternalOutput")

    def bc3(ap2, sz):
        # [P, k] -> [P, k, sz] free-dim broadcast (stride 0)
        return bass.AP(tensor=ap2.tensor, offset=ap2.offset,
                       ap=[ap2.ap[0], ap2.ap[1], [0, sz]])

    with tile.TileContext(nc) as tc, \
         tc.tile_pool(name="const", bufs=1) as constp, \
         tc.tile_pool(name="sbw", bufs=3) as sbw, \
         tc.tile_pool(name="sbt", bufs=3) as sbt, \
         tc.tile_pool(name="psA", bufs=4, space="PSUM") as psA, \
         tc.tile_pool(name="psU", bufs=2, space="PSUM") as psU, \
         tc.tile_pool(name="psO", bufs=2, space="PSUM") as psO:

        xt_t = constp.tile([P, xt_cols], f32)
        nc.sync.dma_start(out=xt_t[:], in_=xt_d[:])
        wk_t = constp.tile([P, r, EMB], f32)
        nc.sync.dma_start(out=wk_t[:], in_=wk_d[:])
        wq_t = constp.tile([P, r, EMB], f32)
        nc.sync.dma_start(out=wq_t[:], in_=wq_d[:])
        wv_t = constp.tile([P, r, EMB], f32)
        nc.sync.dma_start(out=wv_t[:], in_=wv_d[:])
        ut_t = constp.tile([P, r, EMB], f32)
        nc.sync.dma_start(out=ut_t[:], in_=ut_d[:])
        obj_t = constp.tile([P, nt], i32)
        nc.sync.dma_start(out=obj_t[:], in_=obj_d[:])
        rid_t = constp.tile([P, nt], f32)
        nc.sync.dma_start(out=rid_t[:], in_=rid_d[:])
        iota_t = constp.tile([P, P], f32)
        nc.sync.dma_start(out=iota_t[:], in_=iota_d[:])
        id_t = constp.tile([P, P], f32)
        nc.sync.dma_start(out=id_t[:], in_=id_d[:])
        eps_t = constp.tile([P, 1], f32)
        nc.vector.memset(eps_t[:], 1e-30)
        aggnt = constp.tile([P, nwin, P], f32)

        for _it in range(loop_iters):
            _kernel_body(nc, tc, bass, mybir, r, npc, nwpp, tpw,
                         xt_t, wk_t, wq_t, wv_t, ut_t, obj_t, rid_t,
                         iota_t, id_t, eps_t, aggnt, x_d, out_d,
                         sbw, sbt, psA, psU, psO)

    _split_waits(nc, mybir)
    return nc


def _kernel_body(nc, tc, bass, mybir, r, npc, nwpp, tpw,
                 xt_t, wk_t, wq_t, wv_t, ut_t, obj_t, rid_t,
                 iota_t, id_t, eps_t, aggnt, x_d, out_d,
                 sbw, sbt, psA, psU, psO):
    f32 = mybir.dt.float32
    Alu = mybir.AluOpType
    Act = mybir.ActivationFunctionType
    Ax = mybir.AxisListType
    nwin = r * nwpp

    def bc3(ap2, sz):
        return bass.AP(tensor=ap2.tensor, offset=ap2.offset,
                       ap=[ap2.ap[0], ap2.ap[1], [0, sz]])

    if True:
        for w in range(nwin):
            pred = w // nwpp
            sb = w % nwpp
            base = sb * P

            # key projection for this window's 128 subjects
            kwin_ps = psO.tile([P, P], f32, space="PSUM", tag="po")
            nc.tensor.matmul(out=kwin_ps[:], lhsT=xt_t[:, base:base + P],
                             rhs=wk_t[:, pred, :], start=True, stop=True)
            kwin = sbw.tile([P, P], f32, tag="kwin")
            nc.scalar.activation(out=kwin[:], in_=kwin_ps[:], func=Act.Copy,
                                 scale=1.0)

            aggu_ps = psU.tile([P, P + H], f32, space="PSUM", tag="pu")
            for k in range(tpw):
                t = w * tpw + k
                # gather x[obj] for this tile's 128 edges
                xg = sbt.tile([P, P], f32, tag="xg")
                nc.gpsimd.indirect_dma_start(
                    out=xg[:], out_offset=None, in_=x_d[:],
                    in_offset=bass.IndirectOffsetOnAxis(
                        ap=obj_t[:, t:t + 1], axis=0))
                # transpose -> [emb, edges]
                xgT_ps = psA.tile([P, P], f32, space="PSUM", tag="pa")
                nc.tensor.transpose(out=xgT_ps[:], in_=xg[:], identity=id_t[:])
                xgT = sbt.tile([P, P], f32, tag="xgT")
                nc.scalar.activation(out=xgT[:], in_=xgT_ps[:], func=Act.Copy,
                                     scale=1.0)
                # one-hot selector G^T[e, i] = (rid_rel[e] == i)
                GT = sbt.tile([P, P], f32, tag="GT")
                nc.vector.tensor_tensor(
                    out=GT[:], in0=rid_t[:, t:t + 1].to_broadcast([P, P]),
                    in1=iota_t[:], op=Alu.is_equal)
                G_ps = psA.tile([P, P], f32, space="PSUM", tag="pa")
                nc.tensor.transpose(out=G_ps[:], in_=GT[:], identity=id_t[:])
                G = sbt.tile([P, P], f32, tag="G")
                nc.vector.tensor_copy(out=G[:], in_=G_ps[:])
                # projections
                sq_ps = psA.tile([P, P], f32, space="PSUM", tag="pa")
                nc.tensor.matmul(out=sq_ps[:], lhsT=xgT[:],
                                 rhs=wq_t[:, pred, :], start=True, stop=True)
                sq = sbt.tile([P, P], f32, tag="sq")
                nc.vector.tensor_copy(out=sq[:], in_=sq_ps[:])
                sv_ps = psA.tile([P, P], f32, space="PSUM", tag="pa")
                nc.tensor.matmul(out=sv_ps[:], lhsT=xgT[:],
                                 rhs=wv_t[:, pred, :], start=True, stop=True)
                sk_ps = psA.tile([P, P], f32, space="PSUM", tag="pa")
                nc.tensor.matmul(out=sk_ps[:], lhsT=G[:], rhs=kwin[:],
                                 start=True, stop=True)
                # dot product per head, then exp
                prod = sbt.tile([P, P], f32, tag="prod")
                nc.vector.tensor_tensor(out=prod[:], in0=sk_ps[:], in1=sq[:],
                                        op=Alu.mult)
                dot = sbt.tile([P, H], f32, tag="dot")
                nc.vector.tensor_reduce(
                    out=dot[:], in_=prod[:].rearrange("p (h s) -> p h s", h=H),
                    axis=Ax.X, op=Alu.add)
                msg = sbt.tile([P, P + H], f32, tag="msg")
                nc.scalar.activation(out=msg[:, P:P + H], in_=dot[:],
                                     func=Act.Exp, scale=1.0)
                # msg = ex * v  (broadcast ex per head)
                nc.vector.tensor_tensor(
                    out=msg[:, 0:P].rearrange("p (h s) -> p h s", h=H),
                    in0=sv_ps[:].rearrange("p (h s) -> p h s", h=H),
                    in1=bc3(msg[:, P:P + H], S),
                    op=Alu.mult)
                # segment-sum of [msg | ex] into this window's accumulator
                nc.tensor.matmul(out=aggu_ps[:], lhsT=GT[:], rhs=msg[:],
                                 start=(k == 0), stop=(k == tpw - 1))

            # normalize rows: agg / (segsum + eps)
            recip = sbw.tile([P, H], f32, tag="recip")
            nc.scalar.activation(out=recip[:], in_=aggu_ps[:, P:P + H],
                                 func=Act.Copy, bias=1e-30, scale=1.0)
            nc.vector.reciprocal(out=recip[:], in_=recip[:])
            aggn = sbw.tile([P, P], f32, tag="aggn")
            nc.vector.tensor_tensor(
                out=aggn[:].rearrange("p (h s) -> p h s", h=H),
                in0=aggu_ps[:, 0:P].rearrange("p (h s) -> p h s", h=H),
                in1=bc3(recip[:], S),
                op=Alu.mult)
            aggnT_ps = psO.tile([P, P], f32, space="PSUM", tag="po")
            nc.tensor.transpose(out=aggnT_ps[:], in_=aggn[:], identity=id_t[:])
            nc.scalar.activation(out=aggnt[:, w, :], in_=aggnT_ps[:],
                                 func=Act.Copy, scale=1.0)

        # finale: out[n, i] = relu(sum_r aggn[r block] @ unify[r]^T)
        for sb in range(nwpp):
            nrows = min(P, npc - sb * P)
            o_ps = psU.tile([P, P], f32, space="PSUM", tag="pu")
            for pred in range(r):
                nc.tensor.matmul(out=o_ps[:], lhsT=aggnt[:, pred * nwpp + sb, :],
                                 rhs=ut_t[:, pred, :],
                                 start=(pred == 0), stop=(pred == r - 1))
            o_sb = sbw.tile([P, P], f32, tag="osb")
            nc.scalar.activation(out=o_sb[:], in_=o_ps[:], func=Act.Relu,
                                 scale=1.0)
            nc.sync.dma_start(out=out_d[sb * P: sb * P + nrows, :],
                              in_=o_sb[:nrows, :])


def host_prep(x, tokeys, toqueries, tovals, unify, edge_sub, edge_pred,
              edge_obj, n, r, c, npc, nwpp):
    """Shard + pack edges per core; pre-arrange weights. Returns
    (in_maps, tpw)."""
    x = np.ascontiguousarray(np.asarray(x, dtype=np.float32))
    tokeys = np.asarray(tokeys, dtype=np.float32)
    toqueries = np.asarray(toqueries, dtype=np.float32)
    tovals = np.asarray(tovals, dtype=np.float32)
    unify = np.asarray(unify, dtype=np.float32)
    sub = np.asarray(edge_sub).astype(np.int64)
    pred = np.asarray(edge_pred).astype(np.int64)
    obj = np.asarray(edge_obj).astype(np.int64)

    nwin = r * nwpp
    h, s = tokeys.shape[1], tokeys.shape[2]

    def blockdiag(wr):  # [r,h,s,s] -> [emb(j_in), r, emb(i_out)]
        bd = np.zeros((r, EMB, EMB), dtype=np.float32)
        for rr in range(r):
            for hh in range(h):
                # out[n,h,i] = sum_j W[r,h,i,j] x[n,h,j] -> bd[(h,j),(h,i)]=W[r,h,i,j]
                bd[rr, hh * s:(hh + 1) * s, hh * s:(hh + 1) * s] = wr[rr, hh].T
        return np.ascontiguousarray(bd.transpose(1, 0, 2))

    wk_host = blockdiag(tokeys)
    wq_host = blockdiag(toqueries)
    wv_host = blockdiag(tovals)
    ut_host = np.ascontiguousarray(unify.transpose(2, 0, 1))  # [j, r, i]
    iota_host = np.ascontiguousarray(
        np.broadcast_to(np.arange(P, dtype=np.float32), (P, P)))
    id_host = np.eye(P, dtype=np.float32)

    core = sub // npc
    subloc = sub - core * npc
    win = pred * nwpp + subloc // WROWS
    ridrel = (subloc % WROWS).astype(np.float32)

    # per-core packing
    percore = []
    tpw = 1
    for cc in range(c):
        m = core == cc
        wc = win[m]
        order = np.argsort(wc, kind="stable")
        wc = wc[order]
        rr = ridrel[m][order]
        ob = obj[m][order]
        counts = np.bincount(wc, minlength=nwin)
        tpw = max(tpw, int(np.ceil(counts.max() / P)))
        starts = np.zeros(nwin, dtype=np.int64)
        starts[1:] = np.cumsum(counts)[:-1]
        rank = np.arange(len(wc)) - starts[wc]
        percore.append((cc, wc, rr, ob, rank))

    nt = nwin * tpw
    in_maps = []
    for cc, wc, rr, ob, rank in percore:
        slot = wc * (tpw * P) + rank
        obj_arr = np.zeros(nt * P, dtype=np.int32)
        rid_arr = np.full(nt * P, -1.0, dtype=np.float32)
        obj_arr[slot] = ob.astype(np.int32)
        rid_arr[slot] = rr
        obj_host = np.ascontiguousarray(obj_arr.reshape(nt, P).T)
        rid_host = np.ascontiguousarray(rid_arr.reshape(nt, P).T)
        xt_host = np.zeros((EMB, nwpp * P), dtype=np.float32)
        xt_host[:, :npc] = x[cc * npc:(cc + 1) * npc].T
        in_maps.append({
            "x": x, "xt": xt_host,
            "wk": wk_host, "wq": wq_host, "wv": wv_host, "ut": ut_host,
            "obj": obj_host, "rid": rid_host,
            "iota": iota_host, "ident": id_host,
        })
    return in_maps, tpw


_CACHE = {}


def _get_program(n, r, npc, nwpp, tpw):
    key = (n, r, npc, nwpp, tpw)
    if key not in _CACHE:
        _CACHE[key] = build_program(n, r, npc, nwpp, tpw)
    return _CACHE[key]


def kernel(x, tokeys, toqueries, tovals, unify, edge_sub, edge_pred, edge_obj):
    from concourse.bass_utils import run_bass_kernel_spmd

    in_maps, tpw = host_prep(x, tokeys, toqueries, tovals, unify,
                             edge_sub, edge_pred, edge_obj,
                             N, R, C, NPC, NWPP)
    nc = _get_program(N, R, NPC, NWPP, tpw)
    res = run_bass_kernel_spmd(nc, in_maps, list(range(C)))
    out = np.concatenate([res.results[c]["out"] for c in range(C)], axis=0)
    return np.ascontiguousarray(out, dtype=np.float32)
